# revision 20
# baseline (speedup 1.0000x reference)
"""Trainium2 Bass kernel for the BottleneckBlock, data-parallel over batch:
each of 8 NeuronCores computes one batch element end to end (no collectives).

Every matmul runs in fp8e4 DoubleRow (0.5 cycles/row).  Convs use an exact
3-term operand split (W_hi@x_hi + W_lo@x_hi + (W_hi/16)@(16*x_lo)) with
weights prescaled x64 so the fp8 residuals sit in normal range; x splits are
host-side, h2's split costs one extra DVE pass.  InstanceNorm absorbs the
weight prescales (conv1/conv2); the residual-conv path runs at 64x and the
host divides the output by 64 (leaky-relu is positively homogeneous).

Bias algebra: b1/b2 dropped (IN cancels), bk dropped (softmax shift), bq
folded into an augmented k-row (wkT column 32), bv folded into bo on host,
bo/br/negm(-mean*rstd) folded into the matmuls as fp8 constant rows (negm
reaches row-layout via a PE transpose against an fp8 identity).

Attention: q/k at 8x (34-channel [17,2,L] pair layout, ones-row for bq),
P = exp(scores/64 * scale) in fp8, Z via a (1/16)-ones DoubleRow matmul,
o8 = 128*o via reciprocal+partition-broadcast, wo at 8x consuming o8.
"""
import numpy as np
import ml_dtypes

import concourse.bass as bass
import concourse.bacc as bacc
import concourse.mybir as mybir
import concourse.tile as tile
from concourse.bass_utils import run_bass_kernel_spmd

DT = mybir.dt
ALU = mybir.AluOpType
AF = mybir.ActivationFunctionType
DR = mybir.MatmulPerfMode.DoubleRow
BF16 = ml_dtypes.bfloat16
FP8 = ml_dtypes.float8_e4m3

B, C, L = 8, 256, 2048
CR, CO, KW = 32, 512, 5
PAD = KW // 2
LP = L + 2 * PAD
NCH = L // 512             # 512-wide l-chunks
NMT = L // 128             # 128-wide m-tiles
SCALE = CR ** (-0.5)
EPS = 1e-5
SLOPE = 0.2

F_W1 = 3 * 2 * KW * C      # W1h / W1l / W1hd16, [term, i, k, o]

_SEG = {}
_off = 0
for _name, _sz in (("w2", 3 * 2 * KW * CO), ("wr", 3 * 2 * CO),
                   ("wqT", 2 * 64), ("wkT", 2 * 64),
                   ("wvT", 2 * C), ("woT", 2 * C),
                   ("onesz", 2 * 16),
                   ("bo_rows", 2 * 2 * 128), ("ones_bo", 2 * 512),
                   ("br_col", 4), ("qkrows", L)):
    _SEG[_name] = (_off, _off + _sz)
    _off += _sz
F_PK = _off

_CACHED_NC = None


def _build():
    nc = bacc.Bacc("TRN2", target_bir_lowering=False)

    xh_d = nc.dram_tensor("xh", [128, 2, LP], DT.float8e4, kind="ExternalInput")
    xl_d = nc.dram_tensor("xl16", [128, 2, LP], DT.float8e4, kind="ExternalInput")
    w1_d = nc.dram_tensor("w1pack", [128, F_W1], DT.float8e4, kind="ExternalInput")
    wp_d = nc.dram_tensor("wpack", [128, F_PK], DT.float8e4, kind="ExternalInput")
    out_d = nc.dram_tensor("out", [CO, L], DT.bfloat16, kind="ExternalOutput")

    with tile.TileContext(nc) as tc:
        with (
            tc.tile_pool(name="consts", bufs=1) as consts,
            tc.tile_pool(name="big", bufs=1) as big,
            tc.tile_pool(name="ptp", bufs=2) as ptp,
            tc.tile_pool(name="stat", bufs=2) as statp,
            tc.tile_pool(name="small", bufs=8) as smallp,
            tc.tile_pool(name="tmp", bufs=4) as tmpp,
            tc.tile_pool(name="outp", bufs=6) as outp,
        ):
            # ---------------- input DMAs ----------------
            xh = consts.tile([128, 2, LP], DT.float8e4, tag="xh")
            xl = consts.tile([128, 2, LP], DT.float8e4, tag="xl")
            w1all = consts.tile([128, F_W1], DT.float8e4, tag="w1all")
            F3 = F_W1 // 3
            nc.sync.dma_start(out=w1all[:, 0:F3], in_=w1_d[:, 0:F3])
            nc.scalar.dma_start(out=xl[:, :, 0:516], in_=xl_d[:, :, 0:516])
            nc.sync.dma_start(out=xh[:, :, 0:516], in_=xh_d[:, :, 0:516])
            nc.scalar.dma_start(out=w1all[:, F3:2 * F3], in_=w1_d[:, F3:2 * F3])
            nc.sync.dma_start(out=w1all[:, 2 * F3:], in_=w1_d[:, 2 * F3:])
            for _a, _b in ((516, 1028), (1028, LP)):
                nc.sync.dma_start(out=xh[:, :, _a:_b], in_=xh_d[:, :, _a:_b])
                nc.scalar.dma_start(out=xl[:, :, _a:_b], in_=xl_d[:, :, _a:_b])
            wall = consts.tile([128, F_PK], DT.float8e4, tag="wall")
            aw2, bw2 = _SEG["w2"][0], _SEG["wr"][1]
            nc.gpsimd.dma_start(out=wall[:, bw2:], in_=wp_d[:, bw2:])  # small segs first
            nc.gpsimd.dma_start(out=wall[:, aw2:bw2], in_=wp_d[:, aw2:bw2])

            def seg(name, shape=None):
                a, b = _SEG[name]
                t = wall[:, a:b]
                if shape is not None:
                    t = t.rearrange("p (" + " ".join(f"d{i}" for i in range(len(shape)))
                                    + ") -> p " + " ".join(f"d{i}" for i in range(len(shape))),
                                    **{f"d{i}": s for i, s in enumerate(shape[:-1])})
                return t

            w1t = w1all.rearrange("p (t i k o) -> p t i k o", t=3, i=2, k=KW)
            w2t = seg("w2", (3, 2, KW, CO))
            wrt = seg("wr", (3, 2, CO))
            wqt = seg("wqT", (2, 64))
            wkt = seg("wkT", (2, 64))
            wvt = seg("wvT", (2, C))
            wot = seg("woT", (2, C))
            onesz = seg("onesz", (2, 16))[:, :, 0:1]   # [128, 2, 1] = 1/16
            bo_rows = seg("bo_rows", (2, 2, 128))      # [p0][t, s, o]
            ones_bo = seg("ones_bo", (2, 512))         # [p0][s, l]: 1, 1/16
            br_col = seg("br_col")                     # [128, 4] = 64*br fp8

            # conv1 apply multiplier M1 = 16*rsqrt(var') = recip(sqrt(var'/256 + 16*eps))
            # conv2 apply multiplier M2 = 64*rsqrt(var') = recip(sqrt(var'/4096 + eps))
            eps1 = consts.tile([128, 1], DT.float32, tag="eps1")
            nc.vector.memset(eps1, EPS * 4096.0 / 256.0)
            eps2 = consts.tile([128, 1], DT.float32, tag="eps2")
            nc.vector.memset(eps2, EPS * 256.0)

            # persistent activations
            h16 = big.tile([128, 2, L], DT.bfloat16, tag="h16")
            h8 = big.tile([128, 2, L], DT.float8e4, tag="h8")
            h2_16 = big.tile([128, 2, L], DT.bfloat16, tag="h2_16")
            h2h = big.tile([128, 2, LP], DT.float8e4, tag="h2h")
            h2l = big.tile([128, 2, LP], DT.float8e4, tag="h2l")
            for t in (h2h, h2l):
                nc.vector.memset(t[:, :, 0:PAD], 0.0)
                nc.vector.memset(t[:, :, LP - PAD:LP], 0.0)
            q8 = big.tile([17, 2, L], DT.float8e4, tag="q8")
            k8 = big.tile([17, 2, L], DT.float8e4, tag="k8")
            # ones row (q ch32) and zero rows (ch33) via tiny DMAs from wpack
            a0 = _SEG["qkrows"][0]
            nc.sync.dma_start(out=q8[15:17, 1, :], in_=wp_d[15:17, a0:a0 + L])
            nc.sync.dma_start(out=k8[16:17, 1, :], in_=wp_d[16:17, a0:a0 + L])
            vT = big.tile([128, NMT, C], DT.float8e4, tag="vT")
            o8 = big.tile([128, 2, L], DT.float8e4, tag="o8")
            brf = consts.tile([128, 4], DT.float32, tag="brf")
            nc.vector.tensor_copy(brf, br_col)

            def mm(p, lhsT, rhs, first, last):
                nc.tensor.matmul(p, lhsT=lhsT, rhs=rhs, start=first, stop=last,
                                 perf_mode=DR)

            # ---------------- conv1 + IN + leaky ----------------
            with tc.tile_pool(name="psA", bufs=8, space="PSUM") as psA:
                sts = [statp.tile([128, NCH, 6], DT.float32, tag="st1",
                                  name=f"st1{t}") for t in range(2)]
                chunks = {}
                for lc in range(NCH):
                    for t in range(2):
                        osl = slice(t * 128, (t + 1) * 128)
                        p = psA.tile([128, 512], DT.float32, tag="a",
                                     name=f"c1p{t}{lc}")
                        n = 0
                        for wi, rhs in ((0, xh), (1, xh), (2, xl)):
                            for k in range(KW):
                                mm(p, w1t[:, wi, :, k, osl],
                                   rhs[:, :, lc * 512 + k: lc * 512 + k + 512],
                                   n == 0, n == 14)
                                n += 1
                        nc.vector.bn_stats(out=sts[t][:, lc, :], in_=p)
                        chunks[t, lc] = p
                s1s, ngs = [], []
                for t in range(2):
                    mv = smallp.tile([128, 2], DT.float32, tag="mv")
                    s1 = smallp.tile([128, 1], DT.float32, tag="s1",
                                     name=f"s1{t}")
                    ng = smallp.tile([128, 1], DT.float32, tag="ng",
                                     name=f"ng1{t}")
                    nc.vector.bn_aggr(out=mv, in_=sts[t])
                    nc.scalar.activation(out=s1, in_=mv[:, 1:2], func=AF.Sqrt,
                                         bias=eps1, scale=1.0 / 256.0)
                    nc.vector.reciprocal(out=s1, in_=s1)
                    nc.vector.tensor_scalar(out=ng, in0=mv[:, 0:1], scalar1=s1,
                                            scalar2=-1.0, op0=ALU.mult, op1=ALU.mult)
                    s1s.append(s1)
                    ngs.append(ng)
                for lc in range(NCH):
                    lsl = slice(lc * 512, (lc + 1) * 512)
                    for t in range(2):
                        hx = tmpp.tile([128, 512], DT.bfloat16, tag="hx")
                        nc.scalar.activation(out=hx, in_=chunks[t, lc],
                                             func=AF.Identity, bias=ngs[t],
                                             scale=s1s[t])
                        nc.vector.scalar_tensor_tensor(
                            out=h16[:, t, lsl], in0=hx, scalar=SLOPE, in1=hx,
                            op0=ALU.mult, op1=ALU.max)
                    nc.gpsimd.tensor_copy(h8[:, :, lsl], h16[:, :, lsl])

                # ---------------- q, k, vT ----------------
                def qk_make(dst, wt, lc, eng_act):
                    # two 1-bank psums; zero rows (ch33, q ch32) come from the
                    # host-packed zero weight cols; q ones-row via Pool memset.
                    lsl = slice(lc * 512, (lc + 1) * 512)
                    pa = psA.tile([17, 512], DT.float32, tag="a",
                                  name=f"qk{lc}a")
                    pb = psA.tile([17, 512], DT.float32, tag="a",
                                  name=f"qk{lc}b")
                    mm(pa, wt[:, :, 0:17], h8[:, :, lsl], True, True)
                    mm(pb, wt[:, :, 17:34], h8[:, :, lsl], True, True)
                    if eng_act:
                        nc.scalar.copy(out=dst[:, 0, lsl], in_=pa)
                        nc.vector.tensor_copy(dst[:, 1, lsl], pb)
                    else:
                        nc.vector.tensor_copy(dst[:, 0, lsl], pa)
                        nc.scalar.copy(out=dst[:, 1, lsl], in_=pb)
                    if dst is q8:
                        a0q = _SEG["qkrows"][0]
                        nc.sync.dma_start(out=dst[15:16, 1, lsl],
                                          in_=wp_d[15:16, a0q + lc * 512:
                                                   a0q + (lc + 1) * 512])

                for lc in range(NCH):
                    qk_make(k8, wkt, lc, lc % 2 == 0)
                for mt in range(0, NMT, 2):
                    pv = psA.tile([128, 2, C], DT.float32, tag="a",
                                  name=f"vt{mt}")
                    for j in range(2):
                        msl = slice((mt + j) * 128, (mt + j) * 128 + 128)
                        mm(pv[:, j, :], h8[:, :, msl], wvt, True, True)
                    if mt % 4 == 0:
                        nc.scalar.copy(out=vT[:, mt:mt + 2, :], in_=pv)
                    else:
                        nc.vector.tensor_copy(vT[:, mt:mt + 2, :], pv)
                qk_make(q8, wqt, 0, False)

            # ---------------- attention ----------------
            with (
                tc.tile_pool(name="ps2", bufs=2, space="PSUM") as ps2,
                tc.tile_pool(name="psacc", bufs=1, space="PSUM") as psacc,
                tc.tile_pool(name="psz", bufs=1, space="PSUM") as psz,
            ):
                def q_make(lc):
                    lsl = slice(lc * 512, (lc + 1) * 512)
                    pq = ps2.tile([17, 2, 512], DT.float32, tag="s2",
                                  name=f"q{lc}")
                    mm(pq[:, 0, :], wqt[:, :, 0:17], h8[:, :, lsl], True, True)
                    mm(pq[:, 1, :], wqt[:, :, 17:34], h8[:, :, lsl], True, True)
                    nc.vector.tensor_copy(q8[:, :, lsl], pq)
                    a0q = _SEG["qkrows"][0]
                    nc.sync.dma_start(out=q8[15:16, 1, lsl],
                                      in_=wp_d[15:16, a0q + lc * 512:
                                               a0q + (lc + 1) * 512])
                def wo_chunk(lc):
                    lsl = slice(lc * 512, (lc + 1) * 512)
                    for t in range(2):
                        wop = ps2.tile([128, 512], DT.float32, tag="s2",
                                       name=f"wo{t}{lc}")
                        mm(wop, wot[:, :, t * 128:(t + 1) * 128],
                           o8[:, :, lsl], True, False)
                        mm(wop, bo_rows[0:1, t, :, :], ones_bo[0:1, :, :],
                           False, True)
                        nc.vector.scalar_tensor_tensor(
                            out=h2_16[:, t, lsl], in0=wop, scalar=1.0 / 64.0,
                            in1=h16[:, t, lsl], op0=ALU.mult, op1=ALU.add)
                    nc.gpsimd.tensor_copy(
                        h2h[:, :, PAD + lc * 512: PAD + (lc + 1) * 512],
                        h2_16[:, :, lsl])
                    nc.vector.scalar_tensor_tensor(
                        out=h2l[:, :, PAD + lc * 512: PAD + (lc + 1) * 512],
                        in0=h2h[:, :, PAD + lc * 512: PAD + (lc + 1) * 512],
                        scalar=-1.0, in1=h2_16[:, :, lsl],
                        op0=ALU.mult, op1=ALU.add)

                for lc in range(NCH):
                    lsl = slice(lc * 512, (lc + 1) * 512)
                    pt = ptp.tile([128, NMT, 512], DT.float8e4, tag="pt")
                    po = [psacc.tile([128, 512], DT.float32, tag=f"oc{t}",
                                     name=f"oc{t}") for t in range(2)]
                    pz = psz.tile([1, 512], DT.float32, tag="z")
                    for mt in range(0, NMT, 2):
                        mp = slice(mt, mt + 2)
                        sc = ps2.tile([128, 2, 512], DT.float32, tag="s2",
                                      name=f"sc{lc}{mt}")
                        for j in range(2):
                            msl = slice((mt + j) * 128, (mt + j + 1) * 128)
                            mm(sc[:, j, :], k8[:, :, msl], q8[:, :, lsl],
                               True, True)
                        nc.scalar.activation(out=pt[:, mp, :], in_=sc,
                                             func=AF.Exp, scale=SCALE / 256.0)
                        for t in range(2):
                            mm(po[t], vT[:, mp, t * 128:(t + 1) * 128],
                               pt[:, mp, :], mt == 0, mt == NMT - 2)
                        mm(pz, onesz, pt[:, mp, :], mt == 0, mt == NMT - 2)
                    if lc < NCH - 1:
                        q_make(lc + 1)
                    zrec = smallp.tile([1, 512], DT.float32, tag="zrec")
                    nc.vector.reciprocal(out=zrec, in_=pz)
                    bcs = tmpp.tile([128, 512], DT.float32, tag="bcs")
                    nc.gpsimd.partition_broadcast(bcs, zrec)
                    for t in range(2):
                        nc.vector.tensor_tensor(out=o8[:, t, lsl], in0=po[t],
                                                in1=bcs, op=ALU.mult)
                    if lc > 0:
                        wo_chunk(lc - 1)
                wo_chunk(NCH - 1)

            # ---------- conv2 + IN, res conv, leaky, store ----------
            with (
                tc.tile_pool(name="psC", bufs=6, space="PSUM") as psC,
                tc.tile_pool(name="psR", bufs=2, space="PSUM") as psR,
            ):
                for t in range(4):
                    osl = slice(t * 128, (t + 1) * 128)
                    st = statp.tile([128, NCH, 6], DT.float32, tag="st2")
                    chunks = []
                    for lc in range(NCH):
                        p = psC.tile([128, 512], DT.float32, tag="c",
                                     name=f"c2p{t}{lc}")
                        n = 0
                        for k in range(KW):
                            for wi, rhs in ((0, h2h), (1, h2h), (0, h2l)):
                                mm(p, w2t[:, wi, :, k, osl],
                                   rhs[:, :, lc * 512 + k: lc * 512 + k + 512],
                                   n == 0, n == 14)
                                n += 1
                        nc.vector.bn_stats(out=st[:, lc, :], in_=p)
                        chunks.append(p)
                    mv = smallp.tile([128, 2], DT.float32, tag="mv")
                    s2t = smallp.tile([128, 1], DT.float32, tag="s2t")
                    ng = smallp.tile([128, 1], DT.float32, tag="ng")
                    nc.vector.bn_aggr(out=mv, in_=st)
                    nc.scalar.activation(out=s2t, in_=mv[:, 1:2], func=AF.Sqrt,
                                         bias=eps2, scale=1.0 / 4096.0)
                    nc.vector.reciprocal(out=s2t, in_=s2t)
                    nc.vector.tensor_scalar(out=ng, in0=mv[:, 0:1], scalar1=s2t,
                                            scalar2=-1.0, op0=ALU.mult, op1=ALU.mult)
                    for lc in range(NCH):
                        lsl = slice(lc * 512, (lc + 1) * 512)
                        pres = psR.tile([128, 512], DT.float32, tag="r",
                                        name=f"pres{t}{lc}")
                        psl = slice(PAD + lc * 512, PAD + lc * 512 + 512)
                        mm(pres, wrt[:, 0, :, osl], xh[:, :, psl], True, False)
                        mm(pres, wrt[:, 1, :, osl], xh[:, :, psl], False, False)
                        mm(pres, wrt[:, 2, :, osl], xl[:, :, psl], False, True)
                        nsub = 2 if (t == 3 and lc == 3) else 1
                        W = 512 // nsub
                        for sb in range(nsub):
                            ssl = slice(sb * W, (sb + 1) * W)
                            osub = slice(lc * 512 + sb * W,
                                         lc * 512 + (sb + 1) * W)
                            y2a = tmpp.tile([128, W], DT.bfloat16, tag="y2a",
                                            name=f"y2a{t}{lc}{sb}")
                            nc.scalar.activation(out=y2a,
                                                 in_=chunks[lc][:, ssl],
                                                 func=AF.Identity, bias=ng,
                                                 scale=s2t)
                            y2s = tmpp.tile([128, W], DT.bfloat16, tag="y2s",
                                            name=f"y2s{t}{lc}{sb}")
                            nc.vector.scalar_tensor_tensor(
                                out=y2s, in0=y2a, scalar=brf[:, t:t + 1],
                                in1=pres[:, ssl], op0=ALU.add, op1=ALU.add)
                            y02 = tmpp.tile([128, W], DT.bfloat16, tag="y02",
                                            name=f"y02{t}{lc}{sb}")
                            nc.scalar.activation(out=y02, in_=y2s, func=AF.Copy,
                                                 scale=SLOPE)
                            oc = outp.tile([128, W], DT.bfloat16, tag="oc",
                                           name=f"oc{t}{lc}{sb}")
                            nc.vector.tensor_tensor(out=oc, in0=y2s, in1=y02,
                                                    op=ALU.max)
                            engs = (nc.sync, nc.gpsimd, nc.scalar)
                            eng = engs[(t * NCH + lc + sb) % 3] if t == 3 \
                                else engs[(t * NCH + lc) % 2]
                            eng.dma_start(out=out_d[osl, osub], in_=oc)
    nc.finalize()
    return nc


def _get_nc():
    global _CACHED_NC
    if _CACHED_NC is None:
        _CACHED_NC = _build()
    return _CACHED_NC


def _fp8(x):
    return np.asarray(x, np.float32).astype(FP8).astype(np.float32)


def _split_w(ws):
    """ws [O, I, K or none] -> (Wh, Wl, Whd16) fp8-rounded fp32 arrays."""
    wh = _fp8(ws)
    wl = _fp8(ws - wh)
    wd = _fp8(ws / 16.0)
    return wh, wl, wd


def _ik_major(w, K):  # [O, I*K...] -> [128p, 2i, K, O]
    # w: [O, 256, K] -> arr[p, i, k, o] = w[o, i*128+p, k]
    O = w.shape[0]
    return w.reshape(O, 2, 128, K).transpose(2, 1, 3, 0)


def _pack_w1(inputs):
    w1s = 64.0 * inputs["w1"].astype(np.float32)          # [256, 256, 5]
    terms = _split_w(w1s)
    # [128, 3, 2, 5, 256]
    arr = np.stack([_ik_major(t, KW) for t in terms], axis=1)
    return np.ascontiguousarray(arr.reshape(128, -1)).astype(FP8)


def _pack_wp(inputs):
    f = np.float32
    pack = np.zeros((128, F_PK), dtype=f)

    def put(name, a):
        s, e = _SEG[name]
        pack[:, s:e] = a.reshape(128, -1)

    w2s = 64.0 * inputs["w2"].astype(f)
    put("w2", np.stack([_ik_major(t, KW) for t in _split_w(w2s)], axis=1))
    wrs = 64.0 * inputs["wr"].astype(f)                   # [512, 256, 1]
    put("wr", np.stack([_ik_major(t, 1)[:, :, 0, :] for t in _split_w(wrs)],
                       axis=1))

    def qk_cols(w, extra=None):
        # -> [128, 2, 64]: col c<32 = 2*w[c, i*128+p]; col 32 = extra; rest 0
        arr = np.zeros((128, 2, 64), f)
        wt = _fp8(w.astype(f))                            # [32, 256]
        arr[:, :, 0:32] = wt.reshape(32, 2, 128).transpose(2, 1, 0)
        if extra is not None:
            arr[:, :, 32] = _fp8(extra).reshape(2, 128).transpose(1, 0)
        return arr

    wq = inputs["wq"][:, :, 0]
    wk = inputs["wk"][:, :, 0]
    put("wqT", qk_cols(wq))
    put("wkT", qk_cols(wk, 16.0 * (inputs["bq"].astype(f) @ wk.astype(f))))
    put("wvT", _fp8(inputs["wv"][:, :, 0].astype(f))
        .reshape(C, 2, 128).transpose(2, 1, 0))          # [128, 2i, 256c]
    put("woT", _fp8(8.0 * inputs["wo"][:, :, 0].astype(f))
        .reshape(C, 2, 128).transpose(2, 1, 0))
    s, e = _SEG["onesz"]
    pack[:, s:e] = 0.125

    bo_p = inputs["bo"].astype(f) + inputs["wo"][:, :, 0].astype(f) @ \
        inputs["bv"].astype(f)
    bo_hi = _fp8(1024.0 * bo_p)
    bo_lo16 = _fp8(16.0 * (1024.0 * bo_p - bo_hi))
    s, e = _SEG["bo_rows"]                                # [p0][2t, 2s, 128]
    rows = np.zeros((2, 2, 128), f)
    rows[:, 0, :] = bo_hi.reshape(2, 128)
    rows[:, 1, :] = bo_lo16.reshape(2, 128)
    pack[0, s:e] = rows.reshape(-1)
    s, e = _SEG["ones_bo"]                                # [p0][2s, 512]
    ob = np.zeros((2, 512), f)
    ob[0] = 1.0
    ob[1] = 1.0 / 16.0
    pack[0, s:e] = ob.reshape(-1)

    br64 = 64.0 * inputs["br"].astype(f)
    s, e = _SEG["br_col"]                                 # [128, 4t]
    pack[:, s:e] = _fp8(br64).reshape(4, 128).T
    s, e = _SEG["qkrows"]                                 # p15 = 1, p16 = 0
    pack[15, s:e] = 1.0
    return pack.astype(FP8)


def _prep_in_maps(inputs):
    w1pack = _pack_w1(inputs)
    wpack = _pack_wp(inputs)
    x = np.asarray(inputs["x"], dtype=np.float32)         # [B, 256, L]
    xh = _fp8(x)
    xl16 = _fp8(16.0 * (x - xh))

    def shape_x(a):  # [B, 256, LP] -> [B, 128, 2, LP]
        ap = np.pad(a, ((0, 0), (0, 0), (PAD, PAD)))
        return ap.reshape(B, 2, 128, LP).transpose(0, 2, 1, 3)

    xhp = shape_x(xh)
    xlp = shape_x(xl16)
    return [{"w1pack": w1pack, "wpack": wpack,
             "xh": np.ascontiguousarray(xhp[b]).astype(FP8),
             "xl16": np.ascontiguousarray(xlp[b]).astype(FP8)}
            for b in range(B)]


def run(inputs, trace=False):
    nc = _get_nc()
    in_maps = _prep_in_maps(inputs)
    res = run_bass_kernel_spmd(nc, in_maps, core_ids=list(range(B)), trace=trace)
    out = np.stack([np.asarray(res.results[b]["out"]).astype(np.float32)
                    for b in range(B)], axis=0) / 64.0
    return out, res.exec_time_ns


def kernel(**inputs):
    return run(inputs)[0]


# revision 21
# speedup vs baseline: 1.0978x; 1.0978x over previous
"""Trainium2 Bass kernel for the BottleneckBlock, data-parallel over batch:
each of 8 NeuronCores computes one batch element end to end (no collectives).

Every matmul runs in fp8e4 DoubleRow (0.5 cycles/row).  Convs use an exact
3-term operand split (W_hi@x_hi + W_lo@x_hi + (W_hi/16)@(16*x_lo)) with
weights prescaled x64 so the fp8 residuals sit in normal range; x splits are
host-side, h2's split costs one extra DVE pass.  InstanceNorm absorbs the
weight prescales (conv1/conv2); the residual-conv path runs at 64x and the
host divides the output by 64 (leaky-relu is positively homogeneous).

Bias algebra: b1/b2 dropped (IN cancels), bk dropped (softmax shift), bq
folded into an augmented k-row (wkT column 32), bv folded into bo on host,
bo/br/negm(-mean*rstd) folded into the matmuls as fp8 constant rows (negm
reaches row-layout via a PE transpose against an fp8 identity).

Attention: q/k at 8x (34-channel [17,2,L] pair layout, ones-row for bq),
P = exp(scores/64 * scale) in fp8, Z via a (1/16)-ones DoubleRow matmul,
o8 = 128*o via reciprocal+partition-broadcast, wo at 8x consuming o8.
"""
import numpy as np
import ml_dtypes

import concourse.bass as bass
import concourse.bacc as bacc
import concourse.mybir as mybir
import concourse.tile as tile
from concourse.bass_utils import run_bass_kernel_spmd

DT = mybir.dt
ALU = mybir.AluOpType
AF = mybir.ActivationFunctionType
DR = mybir.MatmulPerfMode.DoubleRow
BF16 = ml_dtypes.bfloat16
FP8 = ml_dtypes.float8_e4m3

B, C, L = 8, 256, 2048
CR, CO, KW = 32, 512, 5
PAD = KW // 2
LP = L + 2 * PAD
NCH = L // 512             # 512-wide l-chunks
NMT = L // 128             # 128-wide m-tiles
SCALE = CR ** (-0.5)
EPS = 1e-5
SLOPE = 0.2

F_W1 = 3 * 2 * KW * C      # W1h / W1l / W1hd16, [term, i, k, o]

_SEG = {}
_off = 0
for _name, _sz in (("w2", 3 * 2 * KW * CO), ("wr", 3 * 2 * CO),
                   ("wqT", 2 * 64), ("wkT", 2 * 64),
                   ("wvT", 2 * C), ("woT", 2 * C),
                   ("onesz", 2 * 16),
                   ("bo_rows", 2 * 2 * 128), ("ones_bo", 2 * 512),
                   ("br_col", 4), ("qkrows", L)):
    _SEG[_name] = (_off, _off + _sz)
    _off += _sz
F_PK = _off

_CACHED_NC = None


def _build():
    nc = bacc.Bacc("TRN2", target_bir_lowering=False)

    xh_d = nc.dram_tensor("xh", [128, 2, LP], DT.float8e4, kind="ExternalInput")
    xl_d = nc.dram_tensor("xl16", [128, 2, LP], DT.float8e4, kind="ExternalInput")
    w1_d = nc.dram_tensor("w1pack", [128, F_W1], DT.float8e4, kind="ExternalInput")
    wp_d = nc.dram_tensor("wpack", [128, F_PK], DT.float8e4, kind="ExternalInput")
    out_d = nc.dram_tensor("out", [CO, L], DT.bfloat16, kind="ExternalOutput")

    with tile.TileContext(nc) as tc:
        with (
            tc.tile_pool(name="consts", bufs=1) as consts,
            tc.tile_pool(name="big", bufs=1) as big,
            tc.tile_pool(name="ptp", bufs=2) as ptp,
            tc.tile_pool(name="stat", bufs=2) as statp,
            tc.tile_pool(name="small", bufs=8) as smallp,
            tc.tile_pool(name="tmp", bufs=4) as tmpp,
            tc.tile_pool(name="outp", bufs=6) as outp,
        ):
            # ---------------- input DMAs ----------------
            xh = consts.tile([128, 2, LP], DT.float8e4, tag="xh")
            xl = consts.tile([128, 2, LP], DT.float8e4, tag="xl")
            w1all = consts.tile([128, F_W1], DT.float8e4, tag="w1all")
            F3 = F_W1 // 3
            nc.sync.dma_start(out=w1all[:, 0:F3], in_=w1_d[:, 0:F3])
            nc.scalar.dma_start(out=xl[:, :, 0:516], in_=xl_d[:, :, 0:516])
            nc.sync.dma_start(out=xh[:, :, 0:516], in_=xh_d[:, :, 0:516])
            nc.scalar.dma_start(out=w1all[:, F3:2 * F3], in_=w1_d[:, F3:2 * F3])
            nc.sync.dma_start(out=w1all[:, 2 * F3:], in_=w1_d[:, 2 * F3:])
            for _a, _b in ((516, 1028), (1028, LP)):
                nc.sync.dma_start(out=xh[:, :, _a:_b], in_=xh_d[:, :, _a:_b])
                nc.scalar.dma_start(out=xl[:, :, _a:_b], in_=xl_d[:, :, _a:_b])
            wall = consts.tile([128, F_PK], DT.float8e4, tag="wall")
            aw2, bw2 = _SEG["w2"][0], _SEG["wr"][1]
            nc.gpsimd.dma_start(out=wall[:, bw2:], in_=wp_d[:, bw2:])  # small segs first
            nc.gpsimd.dma_start(out=wall[:, aw2:bw2], in_=wp_d[:, aw2:bw2])

            def seg(name, shape=None):
                a, b = _SEG[name]
                t = wall[:, a:b]
                if shape is not None:
                    t = t.rearrange("p (" + " ".join(f"d{i}" for i in range(len(shape)))
                                    + ") -> p " + " ".join(f"d{i}" for i in range(len(shape))),
                                    **{f"d{i}": s for i, s in enumerate(shape[:-1])})
                return t

            w1t = w1all.rearrange("p (t i k o) -> p t i k o", t=3, i=2, k=KW)
            w2t = seg("w2", (3, 2, KW, CO))
            wrt = seg("wr", (3, 2, CO))
            wqt = seg("wqT", (2, 64))
            wkt = seg("wkT", (2, 64))
            wvt = seg("wvT", (2, C))
            wot = seg("woT", (2, C))
            onesz = seg("onesz", (2, 16))[:, :, 0:1]   # [128, 2, 1] = 1/16
            bo_rows = seg("bo_rows", (2, 2, 128))      # [p0][t, s, o]
            ones_bo = seg("ones_bo", (2, 512))         # [p0][s, l]: 1, 1/16
            br_col = seg("br_col")                     # [128, 4] = 64*br fp8

            # conv1 apply multiplier M1 = 16*rsqrt(var') = recip(sqrt(var'/256 + 16*eps))
            # conv2 apply multiplier M2 = 64*rsqrt(var') = recip(sqrt(var'/4096 + eps))
            eps1 = consts.tile([128, 1], DT.float32, tag="eps1")
            nc.vector.memset(eps1, EPS * 4096.0 / 256.0)
            eps2 = consts.tile([128, 1], DT.float32, tag="eps2")
            nc.vector.memset(eps2, EPS * 256.0)

            # persistent activations
            h16 = big.tile([128, 2, L], DT.bfloat16, tag="h16")
            h8 = big.tile([128, 2, L], DT.float8e4, tag="h8")
            h2_16 = big.tile([128, 2, L], DT.bfloat16, tag="h2_16")
            h2h = big.tile([128, 2, LP], DT.float8e4, tag="h2h")
            h2l = big.tile([128, 2, LP], DT.float8e4, tag="h2l")
            for t in (h2h, h2l):
                nc.vector.memset(t[:, :, 0:PAD], 0.0)
                nc.vector.memset(t[:, :, LP - PAD:LP], 0.0)
            q8 = big.tile([17, 2, L], DT.float8e4, tag="q8")
            k8 = big.tile([17, 2, L], DT.float8e4, tag="k8")
            # ones row (q ch32) and zero rows (ch33) via tiny DMAs from wpack
            a0 = _SEG["qkrows"][0]
            nc.sync.dma_start(out=q8[15:17, 1, :], in_=wp_d[15:17, a0:a0 + L])
            nc.sync.dma_start(out=k8[16:17, 1, :], in_=wp_d[16:17, a0:a0 + L])
            vT = big.tile([128, NMT, C], DT.float8e4, tag="vT")
            o8 = big.tile([128, 2, L], DT.float8e4, tag="o8")
            brf = consts.tile([128, 4], DT.float32, tag="brf")
            nc.vector.tensor_copy(brf, br_col)

            def mm(p, lhsT, rhs, first, last):
                nc.tensor.matmul(p, lhsT=lhsT, rhs=rhs, start=first, stop=last,
                                 perf_mode=DR)

            # ---------------- conv1 + IN + leaky ----------------
            with tc.tile_pool(name="psA", bufs=8, space="PSUM") as psA:
                sts = [statp.tile([128, NCH, 6], DT.float32, tag="st1",
                                  name=f"st1{t}") for t in range(2)]
                chunks = {}
                for lc in range(NCH):
                    for t in range(2):
                        osl = slice(t * 128, (t + 1) * 128)
                        p = psA.tile([128, 512], DT.float32, tag="a",
                                     name=f"c1p{t}{lc}")
                        n = 0
                        for wi, rhs in ((0, xh), (1, xh), (2, xl)):
                            for k in range(KW):
                                mm(p, w1t[:, wi, :, k, osl],
                                   rhs[:, :, lc * 512 + k: lc * 512 + k + 512],
                                   n == 0, n == 14)
                                n += 1
                        nc.vector.bn_stats(out=sts[t][:, lc, :], in_=p)
                        chunks[t, lc] = p
                s1s, ngs = [], []
                for t in range(2):
                    mv = smallp.tile([128, 2], DT.float32, tag="mv")
                    s1 = smallp.tile([128, 1], DT.float32, tag="s1",
                                     name=f"s1{t}")
                    ng = smallp.tile([128, 1], DT.float32, tag="ng",
                                     name=f"ng1{t}")
                    nc.vector.bn_aggr(out=mv, in_=sts[t])
                    nc.scalar.activation(out=s1, in_=mv[:, 1:2], func=AF.Sqrt,
                                         bias=eps1, scale=1.0 / 256.0)
                    nc.vector.reciprocal(out=s1, in_=s1)
                    nc.vector.tensor_scalar(out=ng, in0=mv[:, 0:1], scalar1=s1,
                                            scalar2=-1.0, op0=ALU.mult, op1=ALU.mult)
                    s1s.append(s1)
                    ngs.append(ng)
                for lc in range(NCH):
                    lsl = slice(lc * 512, (lc + 1) * 512)
                    for t in range(2):
                        hx = tmpp.tile([128, 512], DT.bfloat16, tag="hx")
                        nc.scalar.activation(out=hx, in_=chunks[t, lc],
                                             func=AF.Identity, bias=ngs[t],
                                             scale=s1s[t])
                        nc.vector.scalar_tensor_tensor(
                            out=h16[:, t, lsl], in0=hx, scalar=SLOPE, in1=hx,
                            op0=ALU.mult, op1=ALU.max)
                    nc.gpsimd.tensor_copy(h8[:, :, lsl], h16[:, :, lsl])

                # ---------------- q, k, vT ----------------
                def qk_make(dst, wt, lc, eng_act):
                    # two 1-bank psums; zero rows (ch33, q ch32) come from the
                    # host-packed zero weight cols; q ones-row via Pool memset.
                    lsl = slice(lc * 512, (lc + 1) * 512)
                    pa = psA.tile([17, 512], DT.float32, tag="a",
                                  name=f"qk{lc}a")
                    pb = psA.tile([17, 512], DT.float32, tag="a",
                                  name=f"qk{lc}b")
                    mm(pa, wt[:, :, 0:17], h8[:, :, lsl], True, True)
                    mm(pb, wt[:, :, 17:34], h8[:, :, lsl], True, True)
                    nr = 15 if dst is q8 else 17
                    if eng_act:
                        nc.scalar.copy(out=dst[:, 0, lsl], in_=pa)
                        nc.vector.tensor_copy(dst[0:nr, 1, lsl], pb[0:nr, :])
                    else:
                        nc.vector.tensor_copy(dst[:, 0, lsl], pa)
                        nc.scalar.copy(out=dst[0:nr, 1, lsl], in_=pb[0:nr, :])

                for lc in range(NCH):
                    qk_make(k8, wkt, lc, lc % 2 == 0)
                for mt in range(0, 4, 2):
                    pv = psA.tile([128, 2, C], DT.float32, tag="a",
                                  name=f"vt{mt}")
                    for j in range(2):
                        msl = slice((mt + j) * 128, (mt + j) * 128 + 128)
                        mm(pv[:, j, :], h8[:, :, msl], wvt, True, True)
                    if mt % 4 == 0:
                        nc.scalar.copy(out=vT[:, mt:mt + 2, :], in_=pv)
                    else:
                        nc.vector.tensor_copy(vT[:, mt:mt + 2, :], pv)
                qk_make(q8, wqt, 0, False)

            # ---------------- attention ----------------
            with (
                tc.tile_pool(name="ps2", bufs=2, space="PSUM") as ps2,
                tc.tile_pool(name="psacc", bufs=1, space="PSUM") as psacc,
                tc.tile_pool(name="psz", bufs=1, space="PSUM") as psz,
            ):
                def v_make(mt, psv):
                    pv = psv.tile([128, 2, C], DT.float32, tag="v",
                                  name=f"vs{mt}")
                    for j in range(2):
                        msl = slice((mt + j) * 128, (mt + j) * 128 + 128)
                        mm(pv[:, j, :], h8[:, :, msl], wvt, True, True)
                    nc.vector.tensor_copy(vT[:, mt:mt + 2, :], pv)

                def q_make(lc):
                    lsl = slice(lc * 512, (lc + 1) * 512)
                    pq = ps2.tile([17, 2, 512], DT.float32, tag="s2",
                                  name=f"q{lc}")
                    mm(pq[:, 0, :], wqt[:, :, 0:17], h8[:, :, lsl], True, True)
                    mm(pq[:, 1, :], wqt[:, :, 17:34], h8[:, :, lsl], True, True)
                    nc.vector.tensor_copy(q8[:, 0, lsl], pq[:, 0, :])
                    nc.vector.tensor_copy(q8[0:15, 1, lsl], pq[0:15, 1, :])
                def wo_chunk(lc):
                    lsl = slice(lc * 512, (lc + 1) * 512)
                    for t in range(2):
                        wop = ps2.tile([128, 512], DT.float32, tag="s2",
                                       name=f"wo{t}{lc}")
                        mm(wop, wot[:, :, t * 128:(t + 1) * 128],
                           o8[:, :, lsl], True, False)
                        mm(wop, bo_rows[0:1, t, :, :], ones_bo[0:1, :, :],
                           False, True)
                        nc.vector.scalar_tensor_tensor(
                            out=h2_16[:, t, lsl], in0=wop, scalar=1.0 / 64.0,
                            in1=h16[:, t, lsl], op0=ALU.mult, op1=ALU.add)
                    nc.gpsimd.tensor_copy(
                        h2h[:, :, PAD + lc * 512: PAD + (lc + 1) * 512],
                        h2_16[:, :, lsl])
                    nc.vector.scalar_tensor_tensor(
                        out=h2l[:, :, PAD + lc * 512: PAD + (lc + 1) * 512],
                        in0=h2h[:, :, PAD + lc * 512: PAD + (lc + 1) * 512],
                        scalar=-1.0, in1=h2_16[:, :, lsl],
                        op0=ALU.mult, op1=ALU.add)

                for lc in range(NCH):
                    lsl = slice(lc * 512, (lc + 1) * 512)
                    pt = ptp.tile([128, NMT, 512], DT.float8e4, tag="pt")
                    po = [psacc.tile([128, 512], DT.float32, tag=f"oc{t}",
                                     name=f"oc{t}") for t in range(2)]
                    pz = psz.tile([1, 512], DT.float32, tag="z")
                    for mt in range(0, NMT, 2):
                        if lc == 0 and mt >= 2 and mt <= 12:
                            v_make(mt + 2, psz)
                        mp = slice(mt, mt + 2)
                        sc = ps2.tile([128, 2, 512], DT.float32, tag="s2",
                                      name=f"sc{lc}{mt}")
                        for j in range(2):
                            msl = slice((mt + j) * 128, (mt + j + 1) * 128)
                            mm(sc[:, j, :], k8[:, :, msl], q8[:, :, lsl],
                               True, True)
                        nc.scalar.activation(out=pt[:, mp, :], in_=sc,
                                             func=AF.Exp, scale=SCALE / 256.0)
                        for t in range(2):
                            mm(po[t], vT[:, mp, t * 128:(t + 1) * 128],
                               pt[:, mp, :], mt == 0, mt == NMT - 2)
                        mm(pz, onesz, pt[:, mp, :], mt == 0, mt == NMT - 2)
                        if mt == 8 and lc < NCH - 1:
                            q_make(lc + 1)
                    zrec = smallp.tile([1, 512], DT.float32, tag="zrec")
                    nc.vector.reciprocal(out=zrec, in_=pz)
                    bcs = tmpp.tile([128, 512], DT.float32, tag="bcs")
                    nc.gpsimd.partition_broadcast(bcs, zrec)
                    for t in range(2):
                        nc.vector.tensor_tensor(out=o8[:, t, lsl], in0=po[t],
                                                in1=bcs, op=ALU.mult)
                    if lc > 0:
                        wo_chunk(lc - 1)
                wo_chunk(NCH - 1)

            # ---------- conv2 + IN, res conv, leaky, store ----------
            with (
                tc.tile_pool(name="psC", bufs=6, space="PSUM") as psC,
                tc.tile_pool(name="psR", bufs=2, space="PSUM") as psR,
            ):
                for t in range(4):
                    osl = slice(t * 128, (t + 1) * 128)
                    st = statp.tile([128, NCH, 6], DT.float32, tag="st2")
                    chunks = []
                    for lc in range(NCH):
                        p = psC.tile([128, 512], DT.float32, tag="c",
                                     name=f"c2p{t}{lc}")
                        n = 0
                        for k in range(KW):
                            for wi, rhs in ((0, h2h), (1, h2h), (0, h2l)):
                                mm(p, w2t[:, wi, :, k, osl],
                                   rhs[:, :, lc * 512 + k: lc * 512 + k + 512],
                                   n == 0, n == 14)
                                n += 1
                        nc.vector.bn_stats(out=st[:, lc, :], in_=p)
                        chunks.append(p)
                    mv = smallp.tile([128, 2], DT.float32, tag="mv")
                    s2t = smallp.tile([128, 1], DT.float32, tag="s2t")
                    ng = smallp.tile([128, 1], DT.float32, tag="ng")
                    nc.vector.bn_aggr(out=mv, in_=st)
                    nc.scalar.activation(out=s2t, in_=mv[:, 1:2], func=AF.Sqrt,
                                         bias=eps2, scale=1.0 / 4096.0)
                    nc.vector.reciprocal(out=s2t, in_=s2t)
                    nc.vector.tensor_scalar(out=ng, in0=mv[:, 0:1], scalar1=s2t,
                                            scalar2=-1.0, op0=ALU.mult, op1=ALU.mult)
                    for lc in range(NCH):
                        lsl = slice(lc * 512, (lc + 1) * 512)
                        pres = psR.tile([128, 512], DT.float32, tag="r",
                                        name=f"pres{t}{lc}")
                        psl = slice(PAD + lc * 512, PAD + lc * 512 + 512)
                        mm(pres, wrt[:, 0, :, osl], xh[:, :, psl], True, False)
                        mm(pres, wrt[:, 1, :, osl], xh[:, :, psl], False, False)
                        mm(pres, wrt[:, 2, :, osl], xl[:, :, psl], False, True)
                        nsub = 2 if (t == 3 and lc == 3) else 1
                        W = 512 // nsub
                        for sb in range(nsub):
                            ssl = slice(sb * W, (sb + 1) * W)
                            osub = slice(lc * 512 + sb * W,
                                         lc * 512 + (sb + 1) * W)
                            y2a = tmpp.tile([128, W], DT.bfloat16, tag="y2a",
                                            name=f"y2a{t}{lc}{sb}")
                            nc.scalar.activation(out=y2a,
                                                 in_=chunks[lc][:, ssl],
                                                 func=AF.Identity, bias=ng,
                                                 scale=s2t)
                            y2s = tmpp.tile([128, W], DT.bfloat16, tag="y2s",
                                            name=f"y2s{t}{lc}{sb}")
                            nc.vector.scalar_tensor_tensor(
                                out=y2s, in0=y2a, scalar=brf[:, t:t + 1],
                                in1=pres[:, ssl], op0=ALU.add, op1=ALU.add)
                            oc = outp.tile([128, W], DT.bfloat16, tag="oc",
                                           name=f"oc{t}{lc}{sb}")
                            if t == 3 and lc == 3:
                                nc.vector.scalar_tensor_tensor(
                                    out=oc, in0=y2s, scalar=SLOPE, in1=y2s,
                                    op0=ALU.mult, op1=ALU.max)
                            else:
                                y02 = tmpp.tile([128, W], DT.bfloat16,
                                                tag="y02",
                                                name=f"y02{t}{lc}{sb}")
                                nc.scalar.activation(out=y02, in_=y2s,
                                                     func=AF.Copy, scale=SLOPE)
                                nc.vector.tensor_tensor(out=oc, in0=y2s,
                                                        in1=y02, op=ALU.max)
                            engs = (nc.sync, nc.gpsimd, nc.scalar)
                            eng = engs[(t * NCH + lc + sb) % 3] if t == 3 \
                                else engs[(t * NCH + lc) % 2]
                            eng.dma_start(out=out_d[osl, osub], in_=oc)
    nc.finalize()
    return nc


def _get_nc():
    global _CACHED_NC
    if _CACHED_NC is None:
        _CACHED_NC = _build()
    return _CACHED_NC


def _fp8(x):
    return np.asarray(x, np.float32).astype(FP8).astype(np.float32)


def _split_w(ws):
    """ws [O, I, K or none] -> (Wh, Wl, Whd16) fp8-rounded fp32 arrays."""
    wh = _fp8(ws)
    wl = _fp8(ws - wh)
    wd = _fp8(ws / 16.0)
    return wh, wl, wd


def _ik_major(w, K):  # [O, I*K...] -> [128p, 2i, K, O]
    # w: [O, 256, K] -> arr[p, i, k, o] = w[o, i*128+p, k]
    O = w.shape[0]
    return w.reshape(O, 2, 128, K).transpose(2, 1, 3, 0)


def _pack_w1(inputs):
    w1s = 64.0 * inputs["w1"].astype(np.float32)          # [256, 256, 5]
    terms = _split_w(w1s)
    # [128, 3, 2, 5, 256]
    arr = np.stack([_ik_major(t, KW) for t in terms], axis=1)
    return np.ascontiguousarray(arr.reshape(128, -1)).astype(FP8)


def _pack_wp(inputs):
    f = np.float32
    pack = np.zeros((128, F_PK), dtype=f)

    def put(name, a):
        s, e = _SEG[name]
        pack[:, s:e] = a.reshape(128, -1)

    w2s = 64.0 * inputs["w2"].astype(f)
    put("w2", np.stack([_ik_major(t, KW) for t in _split_w(w2s)], axis=1))
    wrs = 64.0 * inputs["wr"].astype(f)                   # [512, 256, 1]
    put("wr", np.stack([_ik_major(t, 1)[:, :, 0, :] for t in _split_w(wrs)],
                       axis=1))

    def qk_cols(w, extra=None):
        # -> [128, 2, 64]: col c<32 = 2*w[c, i*128+p]; col 32 = extra; rest 0
        arr = np.zeros((128, 2, 64), f)
        wt = _fp8(w.astype(f))                            # [32, 256]
        arr[:, :, 0:32] = wt.reshape(32, 2, 128).transpose(2, 1, 0)
        if extra is not None:
            arr[:, :, 32] = _fp8(extra).reshape(2, 128).transpose(1, 0)
        return arr

    wq = inputs["wq"][:, :, 0]
    wk = inputs["wk"][:, :, 0]
    put("wqT", qk_cols(wq))
    put("wkT", qk_cols(wk, 16.0 * (inputs["bq"].astype(f) @ wk.astype(f))))
    put("wvT", _fp8(inputs["wv"][:, :, 0].astype(f))
        .reshape(C, 2, 128).transpose(2, 1, 0))          # [128, 2i, 256c]
    put("woT", _fp8(8.0 * inputs["wo"][:, :, 0].astype(f))
        .reshape(C, 2, 128).transpose(2, 1, 0))
    s, e = _SEG["onesz"]
    pack[:, s:e] = 0.125

    bo_p = inputs["bo"].astype(f) + inputs["wo"][:, :, 0].astype(f) @ \
        inputs["bv"].astype(f)
    bo_hi = _fp8(1024.0 * bo_p)
    bo_lo16 = _fp8(16.0 * (1024.0 * bo_p - bo_hi))
    s, e = _SEG["bo_rows"]                                # [p0][2t, 2s, 128]
    rows = np.zeros((2, 2, 128), f)
    rows[:, 0, :] = bo_hi.reshape(2, 128)
    rows[:, 1, :] = bo_lo16.reshape(2, 128)
    pack[0, s:e] = rows.reshape(-1)
    s, e = _SEG["ones_bo"]                                # [p0][2s, 512]
    ob = np.zeros((2, 512), f)
    ob[0] = 1.0
    ob[1] = 1.0 / 16.0
    pack[0, s:e] = ob.reshape(-1)

    br64 = 64.0 * inputs["br"].astype(f)
    s, e = _SEG["br_col"]                                 # [128, 4t]
    pack[:, s:e] = _fp8(br64).reshape(4, 128).T
    s, e = _SEG["qkrows"]                                 # p15 = 1, p16 = 0
    pack[15, s:e] = 1.0
    return pack.astype(FP8)


def _prep_in_maps(inputs):
    w1pack = _pack_w1(inputs)
    wpack = _pack_wp(inputs)
    x = np.asarray(inputs["x"], dtype=np.float32)         # [B, 256, L]
    xh = _fp8(x)
    xl16 = _fp8(16.0 * (x - xh))

    def shape_x(a):  # [B, 256, LP] -> [B, 128, 2, LP]
        ap = np.pad(a, ((0, 0), (0, 0), (PAD, PAD)))
        return ap.reshape(B, 2, 128, LP).transpose(0, 2, 1, 3)

    xhp = shape_x(xh)
    xlp = shape_x(xl16)
    return [{"w1pack": w1pack, "wpack": wpack,
             "xh": np.ascontiguousarray(xhp[b]).astype(FP8),
             "xl16": np.ascontiguousarray(xlp[b]).astype(FP8)}
            for b in range(B)]


def run(inputs, trace=False):
    nc = _get_nc()
    in_maps = _prep_in_maps(inputs)
    res = run_bass_kernel_spmd(nc, in_maps, core_ids=list(range(B)), trace=trace)
    out = np.stack([np.asarray(res.results[b]["out"]).astype(np.float32)
                    for b in range(B)], axis=0) / 64.0
    return out, res.exec_time_ns


def kernel(**inputs):
    return run(inputs)[0]
